# revision 32
# baseline (speedup 1.0000x reference)
"""Causal attention kernel for Trainium2, 8 NeuronCores.

Problem: x[4,4096,768] f32; Wq/Wk/Wv [768,64] f32.
  q,k,v = x@W*; S = q@k.T (causal); out = softmax(S/8)@v  -> [4,4096,64] f32.

Sharding: data-parallel over batch (4) x interleaved q-chunk split (2).
  The 8 query chunks of 512 rows are split A={0,3,4,7}, B={1,2,5,6};
  both halves get exactly half the causal score area and identical
  exp-instruction counts, so the two programs balance.
  Cores 0-3 run program A (batches 0-3), cores 4-7 run program B.

Device algorithm (per core), v3 (token-major PV, 2-slot proj ring):
  - load xT (host-transposed) [768, NK] bf16 in column waves; slot
    order puts the first kv quarter-wave, the wm constants and the
    phase-1 q waves ahead of everything else so the exp pipeline
    starts as early as the DMA stream allows.
  - projections on PE in bf16 (kv: M=128 [Wv|Wk], q: M=64), 6
    contraction passes per group, into a TWO-slot PSUM ring so the
    next group's projection overlaps this group's consumers.  fp8e4
    e-split DoubleRow operands for S are cast DIRECTLY from PSUM
    (no bf16 staging), one half on DVE and one on gpsimd.
  - v is transposed to token-major ON PE (identity matmul into a bf16
    PSUM tile in the proj ring, Ldweights is free) instead of a DMA
    transpose: the DMA engine mutex is owned by the serial x-wave
    stream for the first ~18us, so a DMA-path transpose would stall
    PV and starve ACT.
  - scores transposed: ST[j,i] per (key tile 128 x q chunk 512) via ONE
    fp8 DoubleRow matmul per key tile (0.5 cyc/row).  fp8 rounding of
    q/k costs ~1.5% rms on the output - the only sub-bf16 step.
  - P = exp(S/8) on ACT to bf16 (no max subtraction: |S/8| <= ~7);
    diagonal 128x128 blocks masked by a triangular multiply (DVE).
  - PV is TOKEN-MAJOR: per (key tile t, token block b) a matmul with
    the P subtile [128 keys, 128 tok] STATIONARY and vx [128 keys, 65]
    moving -> o[128 tok, 65] accumulated in PSUM over t.  Matmul cost
    is output-free-size only, so this halves PV PE time, and the
    softmax denominator lands in PSUM col 64 per block (ones column
    of vx).  Only the chunk's first matmul sets start: psum zeroing
    is per 2KB zero region, later blocks land on pending-zero.
  - normalize: r = 1/denom via reciprocal_approx_fast on the strided
    PSUM denominator columns, then per-partition tensor_scalar_mul
    (DVE) straight from PSUM, then token-major DMA out [512, 64] f32;
    finishes run immediately at chunk completion, and the last
    chunk's finish is split in half so the tail is short.
  - chunks are processed in TWO PHASES of two chunks each with their
    key-tile pairs merged in ascending tile order; q projections are
    emitted at precomputed step indices matching their wave arrival.
  - a dependency-gated chain of tiny warmup matmuls keeps the cost
    model's PE p-state clock running so the real DMA-gated matmuls
    price at ramped speed.
"""

import math

import numpy as np
import ml_dtypes

import concourse.bass as bass
import concourse.bacc as bacc
import concourse.mybir as mybir
import concourse.tile as tile
from concourse.bass_utils import run_bass_kernel_spmd
from concourse.tile_rust import add_dep_helper

B, N, D_IN, D_OUT = 4, 4096, 768, 64
CHUNKS_A = [0, 3, 4, 7]
CHUNKS_B = [1, 2, 5, 6]
NDC = D_IN // 128  # 6 contraction chunks
F8 = mybir.dt.float8e4
BF16 = mybir.dt.bfloat16
F32 = mybir.dt.float32
DR = mybir.MatmulPerfMode.DoubleRow
SCALE = 0.125  # 1/sqrt(64)
WM_W = NDC * 192 + 128 + 64  # [wqkv | mask | I64]
STEP_NS = 1040.0  # ACT exp per full step


def _wave_order(chunks, NK):
    """512-col x-wave order: per chunk (processing order): its q-wave
    first (if not yet loaded), then any kv waves it needs, ascending."""
    waves = []
    loaded = set()

    def add(w):
        if w not in loaded and 512 * w < NK:
            waves.append(w)
            loaded.add(w)

    for c in chunks:
        add(c)  # q-wave = columns [512c, 512c+512)
        for w in range(c + 1):
            add(w)
    return waves


def build_half(chunks, debug_dump=False):
    NQ = 512 * len(chunks)
    T_need = 4 * (max(chunks) + 1)
    NK = 128 * T_need
    nkt = T_need
    nc = bacc.Bacc("TRN2", target_bir_lowering=False, debug=False)

    xT_d = nc.dram_tensor("xT", [D_IN, NK], BF16, kind="ExternalInput")
    # [wqkv 6*192 | mask 128 | I64 64] bf16; wqkv dc-slice layout [Wq|Wv|Wk]
    wm_d = nc.dram_tensor("wm", [128, WM_W], BF16, kind="ExternalInput")
    o_d = nc.dram_tensor("o", [NQ, D_OUT], F32, kind="ExternalOutput")

    from contextlib import ExitStack

    with tile.TileContext(nc) as tc, ExitStack() as stk:
        cpool = stk.enter_context(tc.tile_pool(name="const", bufs=1))
        xpool = stk.enter_context(tc.tile_pool(name="xt", bufs=1))
        jpool = stk.enter_context(tc.tile_pool(name="proj", bufs=1))
        ppool = stk.enter_context(tc.tile_pool(name="pp", bufs=3))
        fpool = stk.enter_context(tc.tile_pool(name="fin", bufs=2))

        # ---- constants / inputs ----
        w_sb = cpool.tile([128, WM_W], BF16, tag="wm")
        w3 = w_sb[:, 0 : NDC * 192].rearrange("p (c j) -> p c j", j=192)
        mask_sb = w_sb[:, NDC * 192 : NDC * 192 + 128]
        i64_sb = w_sb[0:64, NDC * 192 + 128 :]

        zbias = cpool.tile([128, 1], F32, tag="zbias")
        nc.vector.memset(zbias[:, :], 0.0)
        warm_sb = cpool.tile([1, 64], BF16, tag="warm")
        nc.vector.memset(warm_sb[:, :], 0.0)

        xt_sb = xpool.tile([128, NDC * NK], BF16, tag="xt")
        xt3 = xt_sb.rearrange("p (c n) -> p c n", n=NK)
        xT3d = xT_d.ap().rearrange("(c p) n -> p c n", p=128)

        # ---- DMA slot order: [w0a, wm, (phase-1 q waves if not wave 0),
        # w0b, remaining wave order].  Quarter-split of wave 0 lets the
        # first kv group start right after wm. ----
        order = _wave_order(chunks, NK)
        slots = [("x", (0, 256)), ("wm", None)]
        placed = {0}
        if chunks[0] != 0:
            # first chunk's q wave, quarter-split so its q projection can
            # start after the first half lands
            qw = chunks[0]
            slots += [("x", (512 * qw, 256)), ("x", (512 * qw + 256, 256))]
            placed.add(qw)
        slots.append(("x", (256, 256)))
        for w in order:
            if w not in placed:
                slots.append(("x", (512 * w, 512)))
                placed.add(w)
        col_eta = {}
        t_acc = 2300.0
        for kind, payload in slots:
            if kind == "wm":
                t_acc += 940.0
                continue
            g0, g = payload
            t_acc += 2185.0 * g / 512.0
            for c0 in range(g0, g0 + g, 256):
                col_eta[c0] = t_acc + 950.0  # + DMA sem propagation
        for kind, payload in slots:
            if kind == "wm":
                nc.sync.dma_start(w_sb[:, :], wm_d.ap())
            else:
                g0, g = payload
                nc.sync.dma_start(
                    xt3[:, :, g0 : g0 + g], xT3d[:, :, g0 : g0 + g]
                )

        def wave_eta_ms(col):
            # estimated arrival of the wave containing `col` (scheduler
            # hint only: stops the scheduler's DMA-blind model from
            # hoisting projections ahead of attention work)
            return col_eta[(col // 256) * 256] / 1e6

        # per-chunk q emission step index (global step counter) from the
        # arrival model: emit the q projection slightly before its wave
        # lands so the PE picks it up without head-of-line blocking
        q_arrival = {c: col_eta[512 * c + 256] for c in chunks}
        est_first_exp = q_arrival[chunks[0]] + 1700.0
        q_emit_step = {
            c: max(
                0,
                math.ceil((q_arrival[c] - 700.0 - est_first_exp) / STEP_NS),
            )
            for c in chunks
        }
        q_order = sorted(chunks, key=lambda c: q_arrival[c])

        # ---- projection targets ----
        kv_sb = jpool.tile([128, NK], BF16, tag="kv")  # [vT; kT] e-major
        vx_sb = jpool.tile([128, nkt * 65], BF16, tag="vx")
        vx3 = vx_sb.rearrange("p (t e) -> p t e", e=65)
        nc.gpsimd.memset(vx3[:, :, 64:65], 1.0)  # denominator ones column
        kt_sb = jpool.tile([32, nkt * 256], F8, tag="kt")
        kt4 = kt_sb.rearrange("p (t j m) -> p t j m", j=2, m=128)
        qb_sb = jpool.tile([32, NQ], BF16, tag="qb")
        qt_sb = jpool.tile([32, 2 * NQ], F8, tag="qt")
        qt3 = qt_sb.rearrange("p (j n) -> p j n", j=2)

        pref = {}
        done = {"kv": 0, "q": set()}

        def flush_vt():
            # deferred v->token-major transpose of the last kv group (PE
            # identity transpose into the ring, bf16 PSUM out): deferring
            # keeps the ring slot free for a q projection right after the
            # group's staging copy
            pend = done.pop("vt", None)
            if pend is None:
                return
            g0, g = pend
            t0, t1 = g0 // 128, (g0 + g) // 128
            nt = t1 - t0
            vt = pref["proj"].tile([128, nt * 64], BF16, tag="proj", name="vt")
            for j in range(nt):
                nc.tensor.matmul(
                    vt[:, 64 * j : 64 * j + 64],
                    lhsT=kv_sb[0:64, g0 + 128 * j : g0 + 128 * (j + 1)],
                    rhs=i64_sb,
                    is_transpose=True,
                    start=True,
                    stop=True,
                    skip_group_check=True,
                )
            nc.vector.tensor_copy(
                vx3[:, t0:t1, 0:64],
                vt.rearrange("p (t e) -> p t e", e=64),
            )

        def emit_kv_group(g0, g, dep=None):
          flush_vt()
          with tc.tile_wait_until(wave_eta_ms(g0)):
            t0, t1 = g0 // 128, (g0 + g) // 128
            ps = pref["proj"].tile([128, 512], F32, tag="proj", name="pkv")
            for dc in range(NDC):
                mm = nc.tensor.matmul(
                    ps[:, 0:g],
                    lhsT=w3[:, dc, 64:192],
                    rhs=xt3[:, dc, g0 : g0 + g],
                    start=(dc == 0),
                    stop=(dc == NDC - 1),
                )
                if dep is not None and dc == 0:
                    # throttle: stop the scheduler (whose DMA-blind model
                    # thinks projections are ready early) from hoisting
                    # this group ahead of older attention work
                    add_dep_helper(mm.ins, dep.ins, reason="proj throttle")
            # staging copies to SBUF on DVE (release the proj ring
            # slot); k first so the gpsimd fp8 casts start sooner
            nc.vector.tensor_copy(kv_sb[64:128, g0 : g0 + g], ps[64:128, 0:g])
            nc.gpsimd.tensor_copy(kt4[:, t0:t1, 0, :], kv_sb[64:96, g0 : g0 + g])
            nc.gpsimd.tensor_copy(kt4[:, t0:t1, 1, :], kv_sb[96:128, g0 : g0 + g])
            nc.vector.tensor_copy(kv_sb[0:64, g0 : g0 + g], ps[0:64, 0:g])
            done["vt"] = (g0, g)

        def emit_kv_upto(tok, dep=None):
            while done["kv"] < min(tok, NK):
                g0 = done["kv"]
                g = min(256, NK - g0)
                emit_kv_group(g0, g, dep=dep)
                done["kv"] = g0 + g

        def emit_q_half(ci, qc0, h):
          """Project + cast one 256-col half of a q chunk (startup path:
          the half can start as soon as its quarter-wave lands)."""
          with tc.tile_wait_until(wave_eta_ms(qc0 + 256 * h)):
            ql0 = 512 * ci + 256 * h
            ps = pref["proj"].tile([64, 256], F32, tag="proj", name="pqh")
            for dc in range(NDC):
                nc.tensor.matmul(
                    ps[:, :],
                    lhsT=w3[:, dc, 0:64],
                    rhs=xt3[:, dc, qc0 + 256 * h : qc0 + 256 * h + 256],
                    start=(dc == 0),
                    stop=(dc == NDC - 1),
                )
            nc.vector.tensor_copy(qt3[:, 0, ql0 : ql0 + 256], ps[0:32, :])
            nc.vector.tensor_copy(qt3[:, 1, ql0 : ql0 + 256], ps[32:64, :])

        def emit_q(ci, qc0):
          # steady-state q: one full-width projection; staging copy on
          # DVE (releases the ring slot fast), fp8 casts on gpsimd
          if ci in done["q"]:
              return
          done["q"].add(ci)
          with tc.tile_wait_until(wave_eta_ms(qc0 + 256)):
            ql0 = 512 * ci
            ps = pref["proj"].tile([64, 512], F32, tag="proj", name="pq")
            for dc in range(NDC):
                nc.tensor.matmul(
                    ps[:, :],
                    lhsT=w3[:, dc, 0:64],
                    rhs=xt3[:, dc, qc0 : qc0 + 512],
                    start=(dc == 0),
                    stop=(dc == NDC - 1),
                )
            nc.vector.tensor_copy(qt3[:, 0, ql0 : ql0 + 512], ps[0:32, :])
            nc.vector.tensor_copy(qb_sb[:, ql0 : ql0 + 512], ps[32:64, :])
            nc.gpsimd.tensor_copy(
                qt3[:, 1, ql0 : ql0 + 512], qb_sb[:, ql0 : ql0 + 512]
            )

        def q_check(gsi):
            for c in q_order:
                ci = chunks.index(c)
                if ci not in done["q"] and q_emit_step[c] <= gsi:
                    emit_q(ci, 512 * c)

        # ---- psum pools: proj ring 2 + s 2x2 + o 2 = 8 banks ----
        pref["proj"] = stk.enter_context(
            tc.tile_pool(name="ppsum", bufs=2, space="PSUM")
        )
        # p-state warmup chain (see module docstring)
        wsrc = warm_sb
        for wi in range(8):
            warm_ps = pref["proj"].tile([1, 64], F32, tag="proj", name=f"w{wi}")
            nc.tensor.matmul(
                warm_ps[:, :], lhsT=wsrc[:, 0:1], rhs=wsrc[:, :],
                start=True, stop=True,
            )
            wsrc = cpool.tile([1, 64], BF16, tag=f"warm{wi}")
            nc.vector.tensor_copy(wsrc[:, :], warm_ps[:, :])
        spsum = stk.enter_context(tc.tile_pool(name="spsum", bufs=2, space="PSUM"))
        opsum = stk.enter_context(tc.tile_pool(name="opsum", bufs=2, space="PSUM"))

        # ---- attention: two phases, chunks merged by tile order ----
        class Ck:
            def __init__(self, ci, c):
                self.ci, self.c = ci, c
                self.qc0 = 512 * c
                self.ql0 = 512 * ci
                self.T_c = 4 * (c + 1)
                self.npair = self.T_c // 2
                self.o_tile = None
                self.o3 = None
                self.s_cur = None

        def emit_s(ck, pi):
            emit_q(ck.ci, ck.qc0)  # idempotent: q must precede its S
            t0 = 2 * pi
            i0g = max(0, 128 * t0 - ck.qc0)
            s_tile = spsum.tile([128, 1024], F32, tag="s")
            for tl in range(2):
                nc.tensor.matmul(
                    s_tile[:, 512 * tl + i0g : 512 * tl + 512],
                    lhsT=kt4[:, t0 + tl, :, :],
                    rhs=qt3[:, :, ck.ql0 + i0g : ck.ql0 + 512],
                    start=True,
                    stop=True,
                    perf_mode=DR,
                )
            return s_tile

        def emit_s_half(ck, s_half, h):
            # first program step only: scores for one 256-token q half
            # (own [128,512] tile per half so exp h0 has no false dep on
            # the h1 score matmuls)
            for tl in range(2):
                nc.tensor.matmul(
                    s_half[:, 256 * tl : 256 * tl + 256],
                    lhsT=kt4[:, tl, :, :],
                    rhs=qt3[:, :, ck.ql0 + 256 * h : ck.ql0 + 256 * h + 256],
                    start=True,
                    stop=True,
                    perf_mode=DR,
                )

        def emit_fin(ck, blo, bhi):
            """Normalize + store token blocks [blo, bhi) of chunk ck."""
            nb = bhi - blo
            o3 = ck.o3
            r_tile = fpool.tile([128, 4], F32, tag="r", name="r")
            nc.vector.reciprocal_approx_fast(
                r_tile[:, 0:nb], o3[:, blo:bhi, 64]
            )
            n_tile = fpool.tile([128, 256], F32, tag="n", name="n")
            n3 = n_tile.rearrange("p (b e) -> p b e", e=64)
            nc.vector.tensor_tensor(
                n3[:, blo:bhi, :],
                o3[:, blo:bhi, 0:64],
                r_tile[:, 0:nb, None].broadcast_to([128, nb, 64]),
                op=mybir.AluOpType.mult,
            )
            out_ap = o_d.ap()[
                ck.ql0 + 128 * blo : ck.ql0 + 128 * bhi, :
            ].rearrange("(b p) e -> p b e", p=128)
            nc.sync.dma_start(
                out_ap,
                n_tile.rearrange("p (b e) -> p b e", e=64)[:, blo:bhi, :],
            )

        def process(ck, pi, s_next_step):
            """Emit exp/mask/PV for (ck, pi); S for s_next_step emitted
            first so the PE runs ahead of ACT."""
            s_cur = ck.s_cur
            if s_next_step is not None:
                nck, npi = s_next_step
                nck.s_cur = emit_s(nck, npi)
            t0, t1 = 2 * pi, 2 * pi + 1
            i0g = max(0, 128 * t0 - ck.qc0)
            p_tile = ppool.tile([128, 1024], BF16, tag="p")
            p3 = p_tile.rearrange("p (t i) -> p t i", i=512)
            if i0g == 0:
                s_ap, p_ap = s_cur[:, :], p_tile[:, :]
            else:
                s_ap = s_cur.rearrange("p (t i) -> p t i", i=512)[:, :, i0g:512]
                p_ap = p3[:, :, i0g:512]
            exp_inst = nc.scalar.activation(
                p_ap, s_ap, mybir.ActivationFunctionType.Exp,
                bias=zbias[:, :], scale=SCALE,
            )
            for tl, t in ((0, t0), (1, t1)):
                dcol = 128 * t - ck.qc0
                if 0 <= dcol < 512:  # diagonal block: triangular mask
                    nc.vector.tensor_tensor(
                        p3[:, tl, dcol : dcol + 128],
                        p3[:, tl, dcol : dcol + 128],
                        mask_sb[:, :],
                        op=mybir.AluOpType.mult,
                    )
                # token-major PV: P subtile stationary, vx moving.
                # start only on the tile's FIRST matmul: start marks the
                # whole 2KB psum zero region, so later blocks' first
                # writes land on pending-zero (one start per bank).
                b_min = max(0, dcol // 128)
                for b in range(b_min, 4):
                    nc.tensor.matmul(
                        ck.o_tile[:, 65 * b : 65 * b + 65],
                        lhsT=p3[:, tl, 128 * b : 128 * b + 128],
                        rhs=vx3[:, t, :],
                        start=(t == 0 and b == 0),
                        stop=(t == 4 * ck.c + b),
                        skip_group_check=True,
                    )
            return exp_inst

        def process_first_split(ck, s_next_step):
            """First program step, pipelined in two 256-token q halves:
            exp of half 0 runs while half 1's q/S chain completes."""
            s_halves = ck.s_cur
            p_tile = ppool.tile([128, 1024], BF16, tag="p")
            p3 = p_tile.rearrange("p (t i) -> p t i", i=512)
            exps = []
            for h in (0, 1):
                if h == 1 and s_next_step is not None:
                    nck, npi = s_next_step
                    nck.s_cur = emit_s(nck, npi)
                exps.append(
                    nc.scalar.activation(
                        p3[:, :, 256 * h : 256 * h + 256],
                        s_halves[h].rearrange("p (t i) -> p t i", i=256),
                        mybir.ActivationFunctionType.Exp,
                        bias=zbias[:, :],
                        scale=SCALE,
                    )
                )
                for tl, t in ((0, 0), (1, 1)):
                    dcol = 128 * t - ck.qc0
                    if 0 <= dcol < 512 and 256 * h <= dcol < 256 * h + 256:
                        nc.vector.tensor_tensor(
                            p3[:, tl, dcol : dcol + 128],
                            p3[:, tl, dcol : dcol + 128],
                            mask_sb[:, :],
                            op=mybir.AluOpType.mult,
                        )
                    b_min = max(0, dcol // 128)
                    for b in (2 * h, 2 * h + 1):
                        if b < b_min:
                            continue
                        nc.tensor.matmul(
                            ck.o_tile[:, 65 * b : 65 * b + 65],
                            lhsT=p3[:, tl, 128 * b : 128 * b + 128],
                            rhs=vx3[:, t, :],
                            start=(t == 0 and b == 0),
                            stop=(t == 4 * ck.c + b),
                            skip_group_check=True,
                        )
            return exps

        cks = [Ck(ci, c) for ci, c in enumerate(chunks)]
        gsi = 0
        exp_hist = []
        for pidx, phase in enumerate((cks[0:2], cks[2:4])):
            if pidx == 0:
                # sequential: small chunk entirely first — its steps run
                # while the big chunk's q/kv waves are still arriving
                steps = [(ck, pi) for ck in phase for pi in range(ck.npair)]
            else:
                # merged ascending tile order, big chunk lagged 2 pairs
                # (its q wave arrives later); small first on ties
                big = max(phase, key=lambda k: k.npair)
                steps = sorted(
                    [(ck, pi) for ck in phase for pi in range(ck.npair)],
                    key=lambda s: (
                        s[1] + (2 if s[0] is big else 0),
                        s[0].c,
                    ),
                )
            last_ck = max(phase, key=lambda k: k.npair)
            for ck in phase:
                ck.o_tile = opsum.tile(
                    [128, 260], F32, tag="ot", name=f"o{ck.ci}"
                )
                ck.o3 = ck.o_tile.rearrange("p (b e) -> p b e", e=65)

            # prime the first step's S
            emit_kv_upto(128 * (2 * steps[0][1] + 2))
            if pidx == 0:
                ck0 = steps[0][0]
                done["q"].add(ck0.ci)
                emit_q_half(ck0.ci, ck0.qc0, 0)
                s_h0 = spsum.tile([128, 512], F32, tag="s", name="sh0")
                emit_s_half(ck0, s_h0, 0)
                # second kv group before the h1 q chain: its staging
                # copy + casts overlap the h1 projection
                emit_kv_upto(512)
                emit_q_half(ck0.ci, ck0.qc0, 1)
                s_h1 = spsum.tile([128, 512], F32, tag="s", name="sh1")
                emit_s_half(ck0, s_h1, 1)
                ck0.s_cur = (s_h0, s_h1)
            else:
                q_check(gsi)
                steps[0][0].s_cur = emit_s(steps[0][0], steps[0][1])
            for si, (ck, pi) in enumerate(steps):
                nxt = steps[si + 1] if si + 1 < len(steps) else None
                if nxt is not None:
                    # two-step kv lookahead: a group's proj/copy/cast
                    # chain is ~2.5us, more than one exp step
                    la = steps[min(si + 2, len(steps) - 1)]
                    dep = exp_hist[-9] if len(exp_hist) >= 9 else None
                    emit_kv_upto(
                        128 * (2 * max(nxt[1], la[1]) + 2), dep=dep
                    )
                    q_check(gsi)
                flush_vt()
                if pidx == 0 and si == 0:
                    exp_hist.extend(process_first_split(ck, nxt))
                else:
                    exp_hist.append(process(ck, pi, nxt))
                gsi += 1
                if ck is last_ck and ck is cks[-1] and pi == ck.npair - 2:
                    # early half-finish of the final chunk: blocks 0-1
                    # are complete after its second-to-last pair
                    emit_fin(ck, 0, 2)
                    ck.fin_half = True
                if pi == ck.npair - 1:
                    if getattr(ck, "fin_half", False):
                        emit_fin(ck, 2, 4)
                    else:
                        emit_fin(ck, 0, 4)

        if debug_dump:
            kt_d = nc.dram_tensor("kt_dump", [32, nkt * 256], F32, kind="ExternalOutput")
            qt_d = nc.dram_tensor("qt_dump", [32, 2 * NQ], F32, kind="ExternalOutput")
            vx_d = nc.dram_tensor("vx_dump", [128, nkt * 65], F32, kind="ExternalOutput")
            dpool = stk.enter_context(tc.tile_pool(name="dbg", bufs=1))
            ktf = dpool.tile([32, nkt * 256], F32, tag="ktf")
            nc.vector.tensor_copy(ktf[:, :], kt_sb[:, :])
            nc.sync.dma_start(kt_d.ap(), ktf[:, :])
            qtf = dpool.tile([32, 2 * NQ], F32, tag="qtf")
            nc.vector.tensor_copy(qtf[:, :], qt_sb[:, :])
            nc.sync.dma_start(qt_d.ap(), qtf[:, :])
            vxf = dpool.tile([128, nkt * 65], F32, tag="vxf")
            nc.vector.tensor_copy(vxf[:, :], vx_sb[:, :])
            nc.sync.dma_start(vx_d.ap(), vxf[:, :])
    nc.compile()
    return nc


_cache = {}


def _programs():
    if "progs" not in _cache:
        _cache["progs"] = (build_half(CHUNKS_A), build_half(CHUNKS_B))
    return _cache["progs"]


def _host_inputs(x, W_query, W_keys, W_value):
    wqkv = np.concatenate([W_query, W_value, W_keys], axis=1).astype(np.float32)
    i64 = np.zeros((128, 64), np.float32)
    i64[:64, :] = np.eye(64, dtype=np.float32)
    wm = np.concatenate(
        [
            wqkv.reshape(NDC, 128, 192).transpose(1, 0, 2).reshape(128, NDC * 192),
            np.triu(np.ones((128, 128), np.float32)),
            i64,
        ],
        axis=1,
    ).astype(ml_dtypes.bfloat16)
    xT = np.ascontiguousarray(np.transpose(x, (0, 2, 1))).astype(ml_dtypes.bfloat16)
    NK_A = 128 * 4 * (max(CHUNKS_A) + 1)
    NK_B = 128 * 4 * (max(CHUNKS_B) + 1)
    in_A = [
        {"xT": np.ascontiguousarray(xT[b, :, :NK_A]), "wm": wm} for b in range(B)
    ]
    in_B = [
        {"xT": np.ascontiguousarray(xT[b, :, :NK_B]), "wm": wm} for b in range(B)
    ]
    return in_A, in_B


def kernel(x, W_query, W_keys, W_value, _trace=False, _tracedir=None):
    nc_a, nc_b = _programs()
    in_A, in_B = _host_inputs(x, W_query, W_keys, W_value)
    kw = {}
    if _trace:
        kw = dict(trace=True, trace_cores=[0], tmpdir=_tracedir)
    res_a = run_bass_kernel_spmd(nc_a, in_A, core_ids=[0, 1, 2, 3], **kw)
    res_b = run_bass_kernel_spmd(nc_b, in_B, core_ids=[4, 5, 6, 7], **kw)
    out = np.empty((B, N, D_OUT), np.float32)
    for b in range(B):
        for res, chunks in ((res_a, CHUNKS_A), (res_b, CHUNKS_B)):
            for ci, c in enumerate(chunks):
                out[b, 512 * c : 512 * (c + 1)] = res.results[b]["o"][
                    512 * ci : 512 * (ci + 1), :
                ]
    _cache["last_exec_ns"] = (res_a.exec_time_ns, res_b.exec_time_ns)
    return out
